# revision 1
# baseline (speedup 1.0000x reference)
"""GATv2 x3 + MLP (nn_GAT) on trn2, 8 NeuronCores.

v4 design: project per-NODE, gather per-EDGE on host (pure layout).

 - Launch A: project x -> fs1|fd1|res1 per node (block-diag matmuls).
 - Host gathers fs1[src] per edge slot into a node-major padded-ELL
   stream (layout/replication of device-computed bytes only).
 - Launch B: GAT layer-1 pipeline straight from the DMA'd stream:
   add fd, prelu, *attn, score tree (contiguous adds), exp, weighted
   feats, in-place halving trees over slots (contiguous wide runs; DVE
   pays heavily for narrow strided access so every op keeps >=4-wide
   contiguous runs), exact pad-slot denominator fix, residual + elu.
   Epilogue transposes h_def1 per tile on the (otherwise idle) PE and
   projects fs2|fd2|res2 for the d2 layer.
 - Launch C: d2 GAT pipeline (no projection at all) + 14->196->196->
   14->1 MLP on PE.  W2 runs fp8 DoubleRow (two K-rows per pass);
   transposes are batched 2 tiles per instruction via a 32-ch padded
   layout; pure-padding MLP chunks are skipped.

Every FLOP on tensor data runs on device; the host only reorders,
replicates, and dtype-casts bytes.
"""
import sys
sys.path.insert(0, '/opt/trn_rl_repo')
import numpy as np
import ml_dtypes

import concourse.bass as bass
import concourse.mybir as mybir
from concourse import bacc
from concourse.tile import TileContext
from concourse.bass_utils import run_bass_kernel_spmd
from concourse.masks import make_identity

bf16 = mybir.dt.bfloat16
fp8 = mybir.dt.float8e4
f32 = mybir.dt.float32
BF = ml_dtypes.bfloat16
FP8 = ml_dtypes.float8_e4m3
AL = mybir.AluOpType
AF = mybir.ActivationFunctionType
MPM = mybir.MatmulPerfMode

NCORE = 8
P = 128
SUPER = 16          # tiles per supertile
NEG_GAT = 0.2
NEG_MLP = 0.01
FP8_W2 = True


# ================================================================= host prep
def build_schedule(dst, n):
    nloc = n // NCORE
    core_of = dst // nloc
    scheds = []
    for c in range(NCORE):
        em = np.where(core_of == c)[0]
        ldst = dst[em] - c * nloc
        deg = np.bincount(ldst, minlength=nloc)
        nt = -(-nloc // P)
        nt = -(-nt // SUPER) * SUPER
        degp = np.concatenate([deg, np.zeros(nt * P - nloc, np.int64)])
        order = np.argsort(-degp, kind='stable')
        pos_of = np.empty_like(order)
        pos_of[order] = np.arange(len(order))
        scheds.append(dict(core=c, em=em, ldst=ldst, deg=degp, order=order,
                           pos_of=pos_of, nt=nt, nloc=nloc))
    nt = scheds[0]['nt']
    nst = nt // SUPER
    Ls = []
    for st in range(nst):
        L = 1
        for s in scheds:
            L = max(L, int(s['deg'][s['order'][st * SUPER * P]]))
        Ls.append(L)
    return scheds, nst, Ls


def edge_slot_geom(s, Ls):
    order, deg = s['order'], s['deg']
    pos_e = s['pos_of'][s['ldst']]
    eo = np.lexsort((np.arange(len(pos_e)), pos_e))
    pos_sorted = pos_e[eo]
    starts = np.concatenate([[0], np.cumsum(deg[order])])
    rank = np.arange(len(eo)) - starts[pos_sorted]
    t_of = pos_sorted // P
    st_of = t_of // SUPER
    L_e = np.asarray(Ls)[st_of]
    slot = (t_of % SUPER) * L_e + rank
    p_of = pos_sorted % P
    return eo, st_of.astype(np.int64), slot.astype(np.int64), p_of.astype(np.int64)


def pack_G(vals_bf, st_of, slot, p_of, offsC, C, totc):
    buf = np.zeros((P, totc), BF)
    base = offsC[st_of] + slot * C
    for c in range(C):
        buf[p_of, base + c] = vals_bf[:, c]
    return buf


def make_npad(s, Ls, nt):
    L_t = np.repeat(np.asarray(Ls, np.int64), SUPER)
    d = s['deg'][s['order']].reshape(nt, P)
    return np.ascontiguousarray((L_t[:, None] - d).T).astype(np.float32)


def pack_local(vals, nrow, nt):
    pk = np.zeros((8 * nrow, (nt // 8) * P), BF)
    nodes = np.arange(nt * P)
    a = (nodes // P) % 8
    col = (nodes // (8 * P)) * P + nodes % P
    v = vals.astype(BF)
    for f in range(nrow - 1):
        pk[a * nrow + f, col] = v[:, f]
    pk[a * nrow + (nrow - 1), col] = BF(1.0)
    return pk


def blockdiag(w, bias, nrow, sp=16):
    bd = np.zeros((8 * nrow, 8 * sp), np.float32)
    k = w.shape[1]
    for a in range(8):
        bd[a * nrow:a * nrow + w.shape[0], a * sp:a * sp + k] = w
        bd[a * nrow + nrow - 1, a * sp:a * sp + k] = bias
    return bd.astype(BF)


def pm(vals, nt):
    d = vals.shape[1]
    return np.ascontiguousarray(
        vals.reshape(nt, P, d).transpose(1, 0, 2).reshape(P, nt * d))


def l1_colperm_w(a_w, d_w):
    w = np.zeros((5, 12), np.float32)
    w[:, 0:2] = a_w
    for f in range(5):
        for h in range(2):
            w[:, 2 + 2 * f + h] = d_w[:, 5 * h + f]
    return w


def l1_colperm_b(a_b, d_b):
    b = np.zeros(12, np.float32)
    b[0:2] = a_b
    for f in range(5):
        for h in range(2):
            b[2 + 2 * f + h] = d_b[5 * h + f]
    return b


def d2_rowperm(w):
    out = np.zeros_like(w)
    for f in range(5):
        for h in range(2):
            out[2 * f + h] = w[5 * h + f]
    return out


def d2_colperm4(v):
    out = np.zeros_like(v)
    for f in range(2):
        for h in range(2):
            out[..., 2 * f + h] = v[..., 2 * h + f]
    return out


# ================================================================ device bits
def tree_reduce_l(nc, X4, L, outv3):
    """In-place halving tree over the l axis of X4 [P,S,L,C] (destroys X4).
    Final add writes outv3 [P,S,C] f32."""
    cur = L
    while cur > 2:
        h = cur // 2
        nc.vector.tensor_tensor(out=X4[:, :, 0:h, :], in0=X4[:, :, 0:h, :],
                                in1=X4[:, :, cur - h:cur, :], op=AL.add)
        cur -= h
    if cur == 2:
        nc.vector.tensor_tensor(out=outv3.unsqueeze(2), in0=X4[:, :, 0:1, :],
                                in1=X4[:, :, 1:2, :], op=AL.add)
    else:
        nc.vector.tensor_copy(out=outv3.unsqueeze(2), in_=X4[:, :, 0:1, :])


def emit_elu(nc, sbS, hflat, nelem, tag):
    tmp = sbS.tile([P, nelem], f32, tag=tag)
    nc.vector.tensor_scalar_min(out=tmp[:], in0=hflat, scalar1=0.0)
    nc.scalar.activation(out=tmp[:], in_=tmp[:], func=AF.Exp)
    nc.vector.tensor_scalar(out=hflat, in0=hflat, scalar1=0.0, scalar2=-1.0,
                            op0=AL.max, op1=AL.add)
    nc.vector.tensor_tensor(out=hflat, in0=hflat, in1=tmp[:], op=AL.add)


def emit_gat_st(nc, sbG, sbE, sbX, d_g, off, L, S, C, NH, dF, fd_ap, attn_ap,
                h1v, denv, a1):
    """One supertile of the GAT pipeline, (f,h)-interleaved channels."""
    ncols = S * L * C
    G = sbG.tile([P, ncols], bf16, tag="G")
    nc.sync.dma_start(out=G[:], in_=d_g[:, off:off + ncols])
    G4 = G[:].rearrange("p (b l c) -> p b l c", b=S, l=L, c=C)
    E = sbE.tile([P, ncols], bf16, tag="E")
    E4 = E[:].rearrange("p (b l c) -> p b l c", b=S, l=L, c=C)
    fdb = fd_ap.unsqueeze(2).broadcast_to([P, S, L, C])
    nc.vector.tensor_tensor(out=E4, in0=G4, in1=fdb, op=AL.add)
    nc.scalar.activation(out=E4, in_=E4, func=AF.Prelu, alpha=NEG_GAT)
    atb = attn_ap.unsqueeze(1).unsqueeze(1).broadcast_to([P, S, L, C])
    nc.vector.tensor_tensor(out=E4, in0=E4, in1=atb, op=AL.mult)
    # score tree -> channels 0:NH
    if a1:
        nc.vector.tensor_tensor(out=E4[:, :, :, 2:6], in0=E4[:, :, :, 2:6],
                                in1=E4[:, :, :, 6:10], op=AL.add)
        nc.vector.tensor_tensor(out=E4[:, :, :, 2:4], in0=E4[:, :, :, 2:4],
                                in1=E4[:, :, :, 4:6], op=AL.add)
        nc.vector.tensor_tensor(out=E4[:, :, :, 2:4], in0=E4[:, :, :, 2:4],
                                in1=E4[:, :, :, 10:12], op=AL.add)
    else:
        nc.vector.tensor_tensor(out=E4[:, :, :, 0:2], in0=E4[:, :, :, 0:2],
                                in1=E4[:, :, :, 2:4], op=AL.add)
    EX = sbX.tile([P, S * L * NH], bf16, tag="EX")
    EX4 = EX[:].rearrange("p (b l h) -> p b l h", b=S, l=L, h=NH)
    nc.scalar.activation(out=EX4, in_=E4[:, :, :, 0:NH], func=AF.Exp)
    # weighted feats: E <- G * ex  (E dead after exp)
    d0 = NH - 2 if a1 else 0
    if a1:
        nc.vector.tensor_tensor(out=E4[:, :, :, 0:2], in0=G4[:, :, :, 0:2],
                                in1=EX4[:, :, :, 0:2], op=AL.mult)
    c0 = 2 if a1 else 0
    G5 = G4[:, :, :, c0:C].rearrange("p b l (f h) -> p b l f h", f=dF, h=2)
    E5 = E4[:, :, :, c0:C].rearrange("p b l (f h) -> p b l f h", f=dF, h=2)
    exd = EX4[:, :, :, d0:d0 + 2].unsqueeze(3).broadcast_to([P, S, L, dF, 2])
    nc.vector.tensor_tensor(out=E5, in0=G5, in1=exd, op=AL.mult)
    tree_reduce_l(nc, E4, L, h1v)
    tree_reduce_l(nc, EX4, L, denv)


def emit_fixup(nc, sbS, hv, hflat, dv, dflat, fd_ap, rs_flat, np_ap, attn_ap,
               T, C, NH, dF, a1):
    """Pad-slot denominator fix + normalize + residual + elu over T tiles."""
    zp = sbS.tile([P, T * C], bf16, tag="zp")
    zp3 = zp[:].rearrange("p (t c) -> p t c", t=T, c=C)
    nc.scalar.activation(out=zp3, in_=fd_ap, func=AF.Prelu, alpha=NEG_GAT)
    atb = attn_ap.unsqueeze(1).broadcast_to([P, T, C])
    nc.vector.tensor_tensor(out=zp3, in0=zp3, in1=atb, op=AL.mult)
    if a1:
        nc.vector.tensor_tensor(out=zp3[:, :, 2:6], in0=zp3[:, :, 2:6],
                                in1=zp3[:, :, 6:10], op=AL.add)
        nc.vector.tensor_tensor(out=zp3[:, :, 2:4], in0=zp3[:, :, 2:4],
                                in1=zp3[:, :, 4:6], op=AL.add)
        nc.vector.tensor_tensor(out=zp3[:, :, 2:4], in0=zp3[:, :, 2:4],
                                in1=zp3[:, :, 10:12], op=AL.add)
    else:
        nc.vector.tensor_tensor(out=zp3[:, :, 0:2], in0=zp3[:, :, 0:2],
                                in1=zp3[:, :, 2:4], op=AL.add)
    ep = sbS.tile([P, T * NH], bf16, tag="ep")
    ep3 = ep[:].rearrange("p (t h) -> p t h", t=T, h=NH)
    nc.scalar.activation(out=ep3, in_=zp3[:, :, 0:NH], func=AF.Exp)
    padm = sbS.tile([P, T * NH], f32, tag="padm")
    pm3 = padm[:].rearrange("p (t h) -> p t h", t=T, h=NH)
    npb = np_ap.unsqueeze(2).broadcast_to([P, T, NH])
    nc.vector.tensor_tensor(out=pm3, in0=ep3, in1=npb, op=AL.mult)
    nc.vector.tensor_tensor(out=dv, in0=dv, in1=pm3, op=AL.subtract)
    nc.vector.tensor_scalar_max(out=dflat, in0=dflat, scalar1=1e-30)
    rec = sbS.tile([P, T * NH], f32, tag="rec")
    nc.vector.reciprocal(out=rec[:], in_=dflat)
    rec3 = rec[:].rearrange("p (t h) -> p t h", t=T, h=NH)
    d0 = NH - 2 if a1 else 0
    if a1:
        nc.vector.tensor_tensor(out=hv[:, :, 0:2], in0=hv[:, :, 0:2],
                                in1=rec3[:, :, 0:2], op=AL.mult)
    c0 = 2 if a1 else 0
    h5 = hv[:, :, c0:C].rearrange("p t (f h) -> p t f h", f=dF, h=2)
    rb = rec3[:, :, d0:d0 + 2].unsqueeze(2).broadcast_to([P, T, dF, 2])
    nc.vector.tensor_tensor(out=h5, in0=h5, in1=rb, op=AL.mult)
    nc.vector.tensor_tensor(out=hflat, in0=hflat, in1=rs_flat, op=AL.add)
    emit_elu(nc, sbS, hflat, T * C, "elu")


# =============================================================== launch A
def build_launchA(nt):
    cols = nt * 16
    nc = bacc.Bacc("TRN2", target_bir_lowering=False, debug=False, num_devices=NCORE)
    d_x = nc.dram_tensor("x5l", [48, cols], bf16, kind="ExternalInput")
    d_bfs = nc.dram_tensor("bd_fs", [48, P], bf16, kind="ExternalInput")
    d_bfd = nc.dram_tensor("bd_fd", [48, P], bf16, kind="ExternalInput")
    d_brs = nc.dram_tensor("bd_rs", [48, P], bf16, kind="ExternalInput")
    d_fs = nc.dram_tensor("fs1cm", [P, cols], bf16, kind="ExternalOutput")
    d_fd = nc.dram_tensor("fd1cm", [P, cols], bf16, kind="ExternalOutput")
    d_rs = nc.dram_tensor("rs1cm", [P, cols], f32, kind="ExternalOutput")
    with TileContext(nc) as tc:
        with tc.tile_pool(name="res", bufs=1) as res, \
             tc.tile_pool(name="ps", bufs=2, space="PSUM") as ps:
            stg = res.tile([48, cols], bf16)
            nc.sync.dma_start(out=stg[:], in_=d_x[:, :])
            bfs = res.tile([48, P], bf16, tag="bfs")
            nc.sync.dma_start(out=bfs[:], in_=d_bfs[:, :])
            bfd = res.tile([48, P], bf16, tag="bfd")
            nc.sync.dma_start(out=bfd[:], in_=d_bfd[:, :])
            brs = res.tile([48, P], bf16, tag="brs")
            nc.sync.dma_start(out=brs[:], in_=d_brs[:, :])
            ofs = res.tile([P, cols], bf16, tag="ofs")
            ofd = res.tile([P, cols], bf16, tag="ofd")
            ors = res.tile([P, cols], f32, tag="ors")
            k = 0
            for j0 in range(0, cols, 512):
                w = min(512, cols - j0)
                for bd, ot in ((bfs, ofs), (bfd, ofd), (brs, ors)):
                    pmm = ps.tile([P, 512], f32, tag="mm")
                    nc.tensor.matmul(out=pmm[:, :w], lhsT=bd[:], rhs=stg[:, j0:j0 + w],
                                     start=True, stop=True)
                    if k % 2 == 0:
                        nc.vector.tensor_copy(out=ot[:, j0:j0 + w], in_=pmm[:, :w])
                    else:
                        nc.scalar.copy(out=ot[:, j0:j0 + w], in_=pmm[:, :w])
                    k += 1
            nc.sync.dma_start(out=d_fs[:, :], in_=ofs[:])
            nc.sync.dma_start(out=d_fd[:, :], in_=ofd[:])
            nc.sync.dma_start(out=d_rs[:, :], in_=ors[:])
    nc.compile()
    return nc


# =============================================================== launch B
def build_launchB(nst, Ls, offs12, nt):
    totc = int(offs12[-1])
    nc = bacc.Bacc("TRN2", target_bir_lowering=False, debug=False, num_devices=NCORE)
    d_g = nc.dram_tensor("g1", [P, totc], bf16, kind="ExternalInput")
    d_fd = nc.dram_tensor("fd1n", [P, nt * 12], bf16, kind="ExternalInput")
    d_rs = nc.dram_tensor("rs1n", [P, nt * 12], f32, kind="ExternalInput")
    d_at = nc.dram_tensor("attn12", [P, 12], bf16, kind="ExternalInput")
    d_np = nc.dram_tensor("npad", [P, nt], f32, kind="ExternalInput")
    d_w2 = nc.dram_tensor("w2all", [10, 12], bf16, kind="ExternalInput")
    d_bc2 = nc.dram_tensor("bc2", [12], f32, kind="ExternalInput")
    d_f2 = nc.dram_tensor("f2cm", [12, nt * P], f32, kind="ExternalOutput")
    d_ha = nc.dram_tensor("hattn", [P, nt * 2], bf16, kind="ExternalOutput")
    groups = [(0, 5), (5, 9), (9, nst)] if nst >= 9 else [(0, nst)]
    with TileContext(nc) as tc:
        with tc.tile_pool(name="res", bufs=1) as res, \
             tc.tile_pool(name="sbG", bufs=2) as sbG, \
             tc.tile_pool(name="sbE", bufs=2) as sbE, \
             tc.tile_pool(name="sbX", bufs=2) as sbX, \
             tc.tile_pool(name="sbS", bufs=1) as sbS, \
             tc.tile_pool(name="sbT", bufs=2) as sbT, \
             tc.tile_pool(name="psT", bufs=1, space="PSUM") as psT, \
             tc.tile_pool(name="psF", bufs=1, space="PSUM") as psF:
            ident = res.tile([P, P], bf16)
            make_identity(nc, ident[:])
            attn = res.tile([P, 12], bf16)
            nc.sync.dma_start(out=attn[:], in_=d_at[:, :])
            npad = res.tile([P, nt], f32)
            nc.sync.dma_start(out=npad[:], in_=d_np[:, :])
            fdt = res.tile([P, nt * 12], bf16)
            nc.sync.dma_start(out=fdt[:], in_=d_fd[:, :])
            rst = res.tile([P, nt * 12], f32)
            nc.sync.dma_start(out=rst[:], in_=d_rs[:, :])
            w2t = res.tile([10, 12], bf16)
            nc.sync.dma_start(out=w2t[:], in_=d_w2[:, :])
            bc2 = res.tile([12, 1], f32, tag="bc2")
            nc.sync.dma_start(out=bc2[:], in_=d_bc2[:, None])
            h1 = res.tile([P, nt * 12], f32)
            den = res.tile([P, nt * 4], f32)
            hat = res.tile([P, nt * 2], bf16)
            h1v_all = h1[:].rearrange("p (t c) -> p t c", t=nt, c=12)
            denv_all = den[:].rearrange("p (t c) -> p t c", t=nt, c=4)
            fdv_all = fdt[:].rearrange("p (t c) -> p t c", t=nt, c=12)
            npv_all = npad[:].rearrange("p (t o) -> p t o", t=nt, o=1)
            for g0, g1 in groups:
                for st in range(g0, g1):
                    L = Ls[st]
                    t0 = st * SUPER
                    emit_gat_st(nc, sbG, sbE, sbX, d_g, int(offs12[st]), L, SUPER,
                                12, 4, 5, fdv_all[:, t0:t0 + SUPER, :], attn[:],
                                h1v_all[:, t0:t0 + SUPER, :],
                                denv_all[:, t0:t0 + SUPER, :], True)
                T = (g1 - g0) * SUPER
                t0 = g0 * SUPER
                emit_fixup(nc, sbS, h1v_all[:, t0:t0 + T, :],
                           h1[:, t0 * 12:(t0 + T) * 12],
                           denv_all[:, t0:t0 + T, :],
                           den[:, t0 * 4:(t0 + T) * 4],
                           fdv_all[:, t0:t0 + T, :],
                           rst[:, t0 * 12:(t0 + T) * 12],
                           npv_all[:, t0:t0 + T, 0], attn[:], T, 12, 4, 5, True)
                hv = h1v_all[:, t0:t0 + T, :]
                nc.scalar.copy(out=hat[:].rearrange("p (t c) -> p t c", t=nt, c=2)[:, t0:t0 + T, :],
                               in_=hv[:, :, 0:2])
                h1b = sbS.tile([P, T * 12], bf16, tag="h1b")
                nc.scalar.copy(out=h1b[:], in_=h1[:, t0 * 12:(t0 + T) * 12])
                # epilogue: transpose h_def1 per tile, project fs2|fd2|res2
                for st in range(g0, g1):
                    pT = psT.tile([10, SUPER * P], bf16, tag="pT")
                    for b in range(SUPER):
                        trel = (st - g0) * SUPER + b
                        nc.tensor.transpose(out=pT[:, b * P:(b + 1) * P],
                                            in_=h1b[:, trel * 12 + 2:trel * 12 + 12],
                                            identity=ident[:])
                    hT = sbT.tile([10, SUPER * P], bf16, tag="hT")
                    nc.vector.tensor_copy(out=hT[:], in_=pT[:])
                    pF = psF.tile([12, SUPER * P], f32, tag="pF")
                    for q in range(SUPER * P // 512):
                        nc.tensor.matmul(out=pF[:, q * 512:(q + 1) * 512], lhsT=w2t[:],
                                         rhs=hT[:, q * 512:(q + 1) * 512],
                                         start=True, stop=True)
                    # +bias via prelu(alpha=1); Copy forbids AP bias
                    f2s = sbT.tile([12, SUPER * P], f32, tag="f2s")
                    nc.scalar.activation(out=f2s[:], in_=pF[:], func=AF.Prelu,
                                         alpha=1.0, bias=bc2[:])
                    nc.sync.dma_start(out=d_f2[:, st * SUPER * P:(st + 1) * SUPER * P],
                                      in_=f2s[:])
            nc.sync.dma_start(out=d_ha[:, :], in_=hat[:])
    nc.compile()
    return nc


# =============================================================== launch C
def build_launchC(nst, Ls, offs4, nt, nmc):
    totc = int(offs4[-1])
    nc = bacc.Bacc("TRN2", target_bir_lowering=False, debug=False, num_devices=NCORE)
    d_g = nc.dram_tensor("g2", [P, totc], bf16, kind="ExternalInput")
    d_fd = nc.dram_tensor("fd2n", [P, nt * 4], bf16, kind="ExternalInput")
    d_rs = nc.dram_tensor("rs2n", [P, nt * 4], f32, kind="ExternalInput")
    d_at = nc.dram_tensor("attn4", [P, 4], bf16, kind="ExternalInput")
    d_np = nc.dram_tensor("npad", [P, nt], f32, kind="ExternalInput")
    d_ha = nc.dram_tensor("hattn", [P, nt * 2], bf16, kind="ExternalInput")
    d_x = nc.dram_tensor("xpm", [P, nt * 8], bf16, kind="ExternalInput")
    d_w1 = nc.dram_tensor("w1x", [32, 196], bf16, kind="ExternalInput")
    w2dt = fp8 if FP8_W2 else bf16
    d_w2a = nc.dram_tensor("w2dra", [P, 2 * 128], w2dt, kind="ExternalInput")
    d_w2b = nc.dram_tensor("w2drb", [P, 2 * 128], w2dt, kind="ExternalInput")
    d_w3a = nc.dram_tensor("w3a", [P, 14], bf16, kind="ExternalInput")
    d_w3b = nc.dram_tensor("w3b", [68, 14], bf16, kind="ExternalInput")
    d_w4 = nc.dram_tensor("w4", [14, 1], bf16, kind="ExternalInput")
    d_b1 = nc.dram_tensor("b1", [196], f32, kind="ExternalInput")
    d_b2 = nc.dram_tensor("b2", [196], f32, kind="ExternalInput")
    d_b3 = nc.dram_tensor("b3", [14], f32, kind="ExternalInput")
    d_b4 = nc.dram_tensor("b4", [1], f32, kind="ExternalInput")
    d_out = nc.dram_tensor("out", [nmc, 512], f32, kind="ExternalOutput")
    groups = [(0, 3), (3, 6), (6, 9), (9, 11), (11, nst)] if nst >= 11 else [(0, nst)]
    with TileContext(nc) as tc:
        with tc.tile_pool(name="res", bufs=1) as res, \
             tc.tile_pool(name="sbG", bufs=2) as sbG, \
             tc.tile_pool(name="sbE", bufs=2) as sbE, \
             tc.tile_pool(name="sbX", bufs=2) as sbX, \
             tc.tile_pool(name="sbS", bufs=1) as sbS, \
             tc.tile_pool(name="sbM", bufs=2) as sbM, \
             tc.tile_pool(name="psT", bufs=1, space="PSUM") as psT, \
             tc.tile_pool(name="psC", bufs=1, space="PSUM") as psC, \
             tc.tile_pool(name="psA", bufs=2, space="PSUM") as psA, \
             tc.tile_pool(name="psB", bufs=2, space="PSUM") as psB, \
             tc.tile_pool(name="psO", bufs=2, space="PSUM") as psO:
            ident = res.tile([P, P], bf16)
            make_identity(nc, ident[:])
            attn = res.tile([P, 4], bf16)
            nc.sync.dma_start(out=attn[:], in_=d_at[:, :])
            npad = res.tile([P, nt], f32)
            nc.sync.dma_start(out=npad[:], in_=d_np[:, :])
            fdt = res.tile([P, nt * 4], bf16)
            nc.sync.dma_start(out=fdt[:], in_=d_fd[:, :])
            rst = res.tile([P, nt * 4], f32)
            nc.sync.dma_start(out=rst[:], in_=d_rs[:, :])
            hat = res.tile([P, nt * 2], bf16)
            nc.sync.dma_start(out=hat[:], in_=d_ha[:, :])
            xpm = res.tile([P, nt * 8], bf16)
            nc.sync.dma_start(out=xpm[:], in_=d_x[:, :])
            w1 = res.tile([32, 196], bf16, tag="w1")
            nc.sync.dma_start(out=w1[:], in_=d_w1[:, :])
            w2a = res.tile([P, 2 * 128], w2dt, tag="w2a")
            nc.sync.dma_start(out=w2a[:], in_=d_w2a[:, :])
            w2b = res.tile([P, 2 * 128], w2dt, tag="w2b")
            nc.sync.dma_start(out=w2b[:], in_=d_w2b[:, :])
            w3a = res.tile([P, 14], bf16, tag="w3a")
            nc.sync.dma_start(out=w3a[:], in_=d_w3a[:, :])
            w3b = res.tile([68, 14], bf16, tag="w3b")
            nc.sync.dma_start(out=w3b[:], in_=d_w3b[:, :])
            w4 = res.tile([14, 1], bf16, tag="w4")
            nc.sync.dma_start(out=w4[:], in_=d_w4[:, :])
            w2av = w2a[:].rearrange("p (k m) -> p k m", k=2, m=128)
            w2bv = w2b[:].rearrange("p (k m) -> p k m", k=2, m=128)
            b1ca = res.tile([P, 1], f32, tag="b1ca")
            nc.sync.dma_start(out=b1ca[:], in_=d_b1[0:128, None])
            b1cb = res.tile([68, 1], f32, tag="b1cb")
            nc.sync.dma_start(out=b1cb[:], in_=d_b1[128:196, None])
            b2ca = res.tile([P, 1], f32, tag="b2ca")
            nc.sync.dma_start(out=b2ca[:], in_=d_b2[0:128, None])
            b2cb = res.tile([68, 1], f32, tag="b2cb")
            nc.sync.dma_start(out=b2cb[:], in_=d_b2[128:196, None])
            b3c = res.tile([14, 1], f32, tag="b3c")
            nc.sync.dma_start(out=b3c[:], in_=d_b3[:, None])
            b4c = res.tile([1, 1], f32, tag="b4c")
            nc.sync.dma_start(out=b4c[:], in_=d_b4[:, None])
            h2 = res.tile([P, nt * 4], f32)
            den = res.tile([P, nt * 2], f32)
            m32 = res.tile([P, nt * 32], bf16)
            nc.gpsimd.memset(m32[:], 0.0)
            adt = fp8 if FP8_W2 else bf16
            r1t = [res.tile([P, 2 * 512], adt, tag=f"r1_{i}", name=f"r1_{i}")
                   for i in range(2)]
            for t_ in r1t:
                nc.gpsimd.memset(t_[:], 0.0)
            h2v_all = h2[:].rearrange("p (t c) -> p t c", t=nt, c=4)
            denv_all = den[:].rearrange("p (t c) -> p t c", t=nt, c=2)
            fdv_all = fdt[:].rearrange("p (t c) -> p t c", t=nt, c=4)
            npv_all = npad[:].rearrange("p (t o) -> p t o", t=nt, o=1)
            m3_all = m32[:].rearrange("p (t c) -> p t c", t=nt, c=32)
            hav_all = hat[:].rearrange("p (t c) -> p t c", t=nt, c=2)
            xv_all = xpm[:].rearrange("p (t c) -> p t c", t=nt, c=8)
            nc.vector.tensor_copy(out=m3_all[:, :, 0:2], in_=hav_all)
            nc.vector.tensor_copy(out=m3_all[:, :, 6:14], in_=xv_all)
            for g0, g1 in groups:
                for st in range(g0, g1):
                    L = Ls[st]
                    t0 = st * SUPER
                    emit_gat_st(nc, sbG, sbE, sbX, d_g, int(offs4[st]), L, SUPER,
                                4, 2, 2, fdv_all[:, t0:t0 + SUPER, :], attn[:],
                                h2v_all[:, t0:t0 + SUPER, :],
                                denv_all[:, t0:t0 + SUPER, :], False)
                T = (g1 - g0) * SUPER
                t0 = g0 * SUPER
                emit_fixup(nc, sbS, h2v_all[:, t0:t0 + T, :],
                           h2[:, t0 * 4:(t0 + T) * 4],
                           denv_all[:, t0:t0 + T, :],
                           den[:, t0 * 2:(t0 + T) * 2],
                           fdv_all[:, t0:t0 + T, :],
                           rst[:, t0 * 4:(t0 + T) * 4],
                           npv_all[:, t0:t0 + T, 0], attn[:], T, 4, 2, 2, False)
                nc.vector.tensor_copy(out=m3_all[:, t0:t0 + T, 2:6],
                                      in_=h2v_all[:, t0:t0 + T, :])
                # MLP over this group's chunks (4 tiles = 512 nodes each)
                for mc in range(t0 // 4, min((t0 + T) // 4, nmc)):
                    pT = psT.tile([64, 256], bf16, tag="pT")
                    nc.tensor.transpose(out=pT[:, 0:128],
                                        in_=m32[:, (mc * 4) * 32:(mc * 4 + 2) * 32],
                                        identity=ident[:])
                    nc.tensor.transpose(out=pT[:, 128:256],
                                        in_=m32[:, (mc * 4 + 2) * 32:(mc * 4 + 4) * 32],
                                        identity=ident[:])
                    r0 = sbM.tile([32, 512], bf16, tag="r0")
                    nc.vector.tensor_copy(out=r0[:, 0:128], in_=pT[0:32, 0:128])
                    nc.vector.tensor_copy(out=r0[:, 128:256], in_=pT[32:64, 0:128])
                    nc.vector.tensor_copy(out=r0[:, 256:384], in_=pT[0:32, 128:256])
                    nc.vector.tensor_copy(out=r0[:, 384:512], in_=pT[32:64, 128:256])
                    p1a = psA.tile([P, 512], f32, tag="pA")
                    nc.tensor.matmul(out=p1a[:], lhsT=w1[:, 0:128], rhs=r0[:], start=True, stop=True)
                    p1b = psB.tile([P, 512], f32, tag="pB")
                    nc.tensor.matmul(out=p1b[0:68, :], lhsT=w1[:, 128:196], rhs=r0[:], start=True, stop=True)
                    r1 = r1t[mc % 2]
                    r1v = r1[:].rearrange("p (k n) -> p k n", k=2, n=512)
                    nc.scalar.activation(out=r1v[:, 0:1, :], in_=p1a[:].unsqueeze(1),
                                         func=AF.Prelu, alpha=NEG_MLP, bias=b1ca[:])
                    nc.scalar.activation(out=r1v[0:68, 1:2, :], in_=p1b[0:68, :].unsqueeze(1),
                                         func=AF.Prelu, alpha=NEG_MLP, bias=b1cb[:])
                    p2a = psA.tile([P, 512], f32, tag="pA")
                    p2b = psB.tile([P, 512], f32, tag="pB")
                    if FP8_W2:
                        nc.tensor.matmul(out=p2a[:], lhsT=w2av, rhs=r1v,
                                         start=True, stop=True, perf_mode=MPM.DoubleRow)
                        nc.tensor.matmul(out=p2b[:], lhsT=w2bv, rhs=r1v,
                                         start=True, stop=True, perf_mode=MPM.DoubleRow)
                    else:
                        nc.tensor.matmul(out=p2a[:], lhsT=w2av[:, 0, :], rhs=r1v[:, 0, :], start=True, stop=False)
                        nc.tensor.matmul(out=p2a[:], lhsT=w2av[0:68, 1, :], rhs=r1v[0:68, 1, :], start=False, stop=True)
                        nc.tensor.matmul(out=p2b[:], lhsT=w2bv[:, 0, :], rhs=r1v[:, 0, :], start=True, stop=False)
                        nc.tensor.matmul(out=p2b[:], lhsT=w2bv[0:68, 1, :], rhs=r1v[0:68, 1, :], start=False, stop=True)
                    r2a = sbM.tile([P, 512], bf16, tag="r2a")
                    nc.scalar.activation(out=r2a[:], in_=p2a[:], func=AF.Prelu,
                                         alpha=NEG_MLP, bias=b2ca[:])
                    r2b = sbM.tile([68, 512], bf16, tag="r2b")
                    nc.scalar.activation(out=r2b[:], in_=p2b[0:68, :], func=AF.Prelu,
                                         alpha=NEG_MLP, bias=b2cb[:])
                    p3 = psC.tile([14, 512], f32, tag="p3")
                    nc.tensor.matmul(out=p3[:], lhsT=w3a[:], rhs=r2a[:], start=True, stop=False)
                    nc.tensor.matmul(out=p3[:], lhsT=w3b[:], rhs=r2b[:], start=False, stop=True)
                    r3 = sbM.tile([14, 512], bf16, tag="r3")
                    nc.scalar.activation(out=r3[:], in_=p3[:], func=AF.Prelu,
                                         alpha=NEG_MLP, bias=b3c[:])
                    po = psO.tile([1, 512], f32, tag="po")
                    nc.tensor.matmul(out=po[:], lhsT=w4[:], rhs=r3[:],
                                     start=True, stop=True)
                    sg = sbM.tile([1, 512], f32, tag="sg")
                    nc.scalar.activation(out=sg[:], in_=po[:], func=AF.Sigmoid,
                                         bias=b4c[:])
                    nc.sync.dma_start(out=d_out[mc:mc + 1, :], in_=sg[:])
    nc.compile()
    return nc


# ================================================================== kernel
_cache = {}


def kernel(**inputs):
    x = np.asarray(inputs['x'], np.float32)
    src = np.asarray(inputs['src'], np.int32)
    dst = np.asarray(inputs['dst'], np.int32)
    n = x.shape[0]

    scheds, nst, Ls = build_schedule(dst, n)
    nt = scheds[0]['nt']
    nloc = scheds[0]['nloc']
    nmc = -(-nloc // 512)          # MLP chunks covering real nodes only
    offs12 = np.concatenate([[0], np.cumsum([SUPER * L * 12 for L in Ls])]).astype(np.int64)
    offs4 = np.concatenate([[0], np.cumsum([SUPER * L * 4 for L in Ls])]).astype(np.int64)

    bd_fs = blockdiag(l1_colperm_w(np.asarray(inputs['a1_Wsrc']), np.asarray(inputs['d1_Wsrc'])),
                      l1_colperm_b(np.asarray(inputs['a1_bsrc']), np.asarray(inputs['d1_bsrc'])), 6)
    bd_fd = blockdiag(l1_colperm_w(np.asarray(inputs['a1_Wdst']), np.asarray(inputs['d1_Wdst'])),
                      l1_colperm_b(np.asarray(inputs['a1_bdst']), np.asarray(inputs['d1_bdst'])), 6)
    bd_rs = blockdiag(l1_colperm_w(np.asarray(inputs['a1_Wres']), np.asarray(inputs['d1_Wres'])),
                      l1_colperm_b(np.asarray(inputs['a1_bias']), np.asarray(inputs['d1_bias'])), 6)
    attn12 = np.zeros(12, np.float32)
    attn12[0:2] = np.asarray(inputs['a1_attn'])[:, 0]
    for f in range(5):
        for h in range(2):
            attn12[2 + 2 * f + h] = np.asarray(inputs['d1_attn'])[h, f]
    attn12_t = np.tile(attn12.astype(BF), (P, 1))

    ws2 = d2_rowperm(d2_colperm4(np.asarray(inputs['d2_Wsrc'], np.float32)))
    bs2 = d2_colperm4(np.asarray(inputs['d2_bsrc'], np.float32))
    wd2 = d2_rowperm(d2_colperm4(np.asarray(inputs['d2_Wdst'], np.float32)))
    bdst2 = d2_colperm4(np.asarray(inputs['d2_bdst'], np.float32))
    wr2 = d2_rowperm(d2_colperm4(np.asarray(inputs['d2_Wres'], np.float32)))
    bias2 = d2_colperm4(np.asarray(inputs['d2_bias'], np.float32))
    w2all = np.concatenate([ws2, wd2, wr2], axis=1)                  # [10, 12]
    bc2 = np.concatenate([bs2, bdst2, bias2]).astype(np.float32)
    attn4 = np.zeros(4, np.float32)
    for f in range(2):
        for h in range(2):
            attn4[2 * f + h] = np.asarray(inputs['d2_attn'])[h, f]
    attn4_t = np.tile(attn4.astype(BF), (P, 1))

    w1p = np.asarray(inputs['W1'], np.float32).copy()
    for f in range(2):
        for h in range(2):
            w1p[2 + 2 * f + h] = np.asarray(inputs['W1'])[2 + 2 * h + f]
    w1x = np.zeros((32, 196), np.float32)
    w1x[0:14] = w1p
    W2 = np.asarray(inputs['W2'], np.float32)
    w2dra = np.zeros((P, 2, 128), np.float32)
    w2dra[:, 0, :] = W2[0:128, 0:128]
    w2dra[0:68, 1, :] = W2[128:196, 0:128]
    w2drb = np.zeros((P, 2, 128), np.float32)
    w2drb[:, 0, 0:68] = W2[0:128, 128:196]
    w2drb[0:68, 1, 0:68] = W2[128:196, 128:196]
    FPW = FP8 if FP8_W2 else BF

    key = (n, len(src), nst, tuple(Ls))
    if key not in _cache:
        _cache[key] = (build_launchA(nt), build_launchB(nst, Ls, offs12, nt),
                       build_launchC(nst, Ls, offs4, nt, nmc))
    ncA, ncB, ncC = _cache[key]

    # ---------------- launch A: per-node projections of x
    inA = []
    for s in scheds:
        orig = s['order']
        valid = orig < nloc
        xl = np.zeros((nt * P, 5), np.float32)
        xl[valid] = x[s['core'] * nloc + orig[valid], :5]
        inA.append(dict(x5l=pack_local(xl, 6, nt), bd_fs=bd_fs, bd_fd=bd_fd, bd_rs=bd_rs))
    rA = run_bass_kernel_spmd(ncA, inA, core_ids=list(range(NCORE)))
    tA = rA.exec_time_ns or 0

    i_all = np.arange(nt * P)
    a_i = (i_all // P) % 8
    col_i = (i_all // (8 * P)) * P + i_all % P
    rows12 = a_i[:, None] * 16 + np.arange(12)[None, :]
    fs1g = np.zeros((n, 12), BF)
    geoms, fd1n_l, rs1n_l, npad_l = [], [], [], []
    for ci, s in enumerate(scheds):
        fs_sorted = rA.results[ci]['fs1cm'][rows12, col_i[:, None]]
        fd_sorted = rA.results[ci]['fd1cm'][rows12, col_i[:, None]]
        rs_sorted = rA.results[ci]['rs1cm'][rows12, col_i[:, None]]
        orig = s['order']
        valid = orig < nloc
        fs1g[s['core'] * nloc + orig[valid]] = fs_sorted[valid]
        fd1n_l.append(pm(fd_sorted, nt))
        rs1n_l.append(pm(rs_sorted.astype(np.float32), nt))
        geoms.append(edge_slot_geom(s, Ls))
        npad_l.append(make_npad(s, Ls, nt))

    inB = []
    for ci, s in enumerate(scheds):
        eo, st_of, slot, p_of = geoms[ci]
        v = fs1g[src[s['em']][eo]]
        g1 = pack_G(v, st_of, slot, p_of, offs12, 12, int(offs12[-1]))
        inB.append(dict(g1=g1, fd1n=fd1n_l[ci], rs1n=rs1n_l[ci], attn12=attn12_t,
                        npad=npad_l[ci], w2all=w2all.astype(BF), bc2=bc2))
    rB = run_bass_kernel_spmd(ncB, inB, core_ids=list(range(NCORE)))
    tB = rB.exec_time_ns or 0

    fs2g = np.zeros((n, 4), BF)
    fd2n_l, rs2n_l, ha_l, xpm_l = [], [], [], []
    for ci, s in enumerate(scheds):
        f2 = rB.results[ci]['f2cm']          # [12, nt*P] f32, col = sorted idx
        orig = s['order']
        valid = orig < nloc
        fs2g[s['core'] * nloc + orig[valid]] = f2[0:4, :].T[valid].astype(BF)
        fd2n_l.append(pm(f2[4:8, :].T.astype(BF), nt))
        rs2n_l.append(pm(np.ascontiguousarray(f2[8:12, :].T), nt))
        ha_l.append(rB.results[ci]['hattn'])
        xl8 = np.zeros((nt * P, 8), np.float32)
        xl8[valid] = x[s['core'] * nloc + orig[valid], :]
        xpm_l.append(pm(xl8, nt).astype(BF))

    inC = []
    for ci, s in enumerate(scheds):
        eo, st_of, slot, p_of = geoms[ci]
        v = fs2g[src[s['em']][eo]]
        g2 = pack_G(v, st_of, slot, p_of, offs4, 4, int(offs4[-1]))
        inC.append(dict(g2=g2, fd2n=fd2n_l[ci], rs2n=rs2n_l[ci], attn4=attn4_t,
                        npad=npad_l[ci], hattn=ha_l[ci], xpm=xpm_l[ci],
                        w1x=w1x.astype(BF),
                        w2dra=w2dra.reshape(P, 256).astype(FPW),
                        w2drb=w2drb.reshape(P, 256).astype(FPW),
                        w3a=np.asarray(inputs['W3'], np.float32)[0:128].astype(BF),
                        w3b=np.asarray(inputs['W3'], np.float32)[128:196].astype(BF),
                        w4=np.asarray(inputs['W4'], np.float32).astype(BF),
                        b1=np.asarray(inputs['b1'], np.float32),
                        b2=np.asarray(inputs['b2'], np.float32),
                        b3=np.asarray(inputs['b3'], np.float32),
                        b4=np.asarray(inputs['b4'], np.float32)))
    rC = run_bass_kernel_spmd(ncC, inC, core_ids=list(range(NCORE)))
    tC = rC.exec_time_ns or 0

    out = np.zeros((n, 1), np.float32)
    for ci, s in enumerate(scheds):
        y = rC.results[ci]['out'].reshape(nmc * 512)
        orig = s['order']
        valid = orig < nloc
        idx = np.arange(nt * P)[valid]
        out[s['core'] * nloc + orig[valid], 0] = y[idx]
    kernel.last_exec_ns = tA + tB + tC
    kernel.last_t12 = (tA, tB, tC)
    kernel.last_results = (rA, rB, rC)
    return out



# revision 8
# speedup vs baseline: 1.0191x; 1.0191x over previous
"""GATv2 x3 + MLP (nn_GAT) on trn2, 8 NeuronCores.

v5 design: attn folded into projection weights (leaky_relu is positively
homogeneous; negative attn handled by a slope-5 prelu + per-node unscale),
l-innermost edge stream so every DVE op runs in 2x mode, den-reduce on the
Pool engine, MLP tail batched (stacked 14->1 matmul + one sigmoid pass).

 - Launch A: project x -> Y|fd''|res per node (block-diag matmuls);
   Y = attn-scaled source projection, fd'' = attn-scaled dest projection.
 - Host gathers Y[src] per edge into a node-major padded-ELL stream with
   edge slots INNERMOST: [P, S, C, L].
 - Launch B: GAT layer-1: z = Y[src]+fd''[dst] (pair-trick broadcast keeps
   2x), prelu with per-sign-run alphas, score tree (4 wide strided adds),
   exp, weighted feats in-place, halving reduce over slots; den reduce on
   gpsimd.  Fixup: pad-slot den fix, normalize, unscale, residual, elu.
   Epilogue projects fs2''|fd2''|res2 on the otherwise idle PE.
 - Launch C: d2 GAT pipeline + 14->196->196->14->1 MLP.  W2 fp8 DoubleRow;
   r3 staged so the 14->1 matmul runs 8 chunks per instruction and all
   sigmoids run in one table-load at the end.

Host only reorders/replicates/casts device-computed tensor bytes; the only
host arithmetic is on the tiny weight matrices (attn folding).
"""
import sys
sys.path.insert(0, '/opt/trn_rl_repo')
import numpy as np
import ml_dtypes

import concourse.bass as bass
import concourse.mybir as mybir
from concourse import bacc
from concourse.tile import TileContext
from concourse.bass_utils import run_bass_kernel_spmd
from concourse.masks import make_identity

bf16 = mybir.dt.bfloat16
fp8 = mybir.dt.float8e4
f32 = mybir.dt.float32
BF = ml_dtypes.bfloat16
FP8 = ml_dtypes.float8_e4m3
AL = mybir.AluOpType
AF = mybir.ActivationFunctionType
MPM = mybir.MatmulPerfMode

NCORE = 8
P = 128
SUPER = 16          # tiles per supertile
NEG_GAT = 0.2
NEG_MLP = 0.01
FP8_W2 = True


# ================================================================= host prep
def build_schedule(dst, n):
    nloc = n // NCORE
    core_of = dst // nloc
    scheds = []
    for c in range(NCORE):
        em = np.where(core_of == c)[0]
        ldst = dst[em] - c * nloc
        deg = np.bincount(ldst, minlength=nloc)
        nt = -(-nloc // P)
        nt = -(-nt // SUPER) * SUPER
        degp = np.concatenate([deg, np.zeros(nt * P - nloc, np.int64)])
        order = np.argsort(-degp, kind='stable')
        pos_of = np.empty_like(order)
        pos_of[order] = np.arange(len(order))
        scheds.append(dict(core=c, em=em, ldst=ldst, deg=degp, order=order,
                           pos_of=pos_of, nt=nt, nloc=nloc))
    nt = scheds[0]['nt']
    nst = nt // SUPER
    Ls = []
    for st in range(nst):
        L = 2
        for s in scheds:
            L = max(L, int(s['deg'][s['order'][st * SUPER * P]]))
        L += L & 1          # pair-trick needs even L
        Ls.append(L)
    return scheds, nst, Ls


def edge_slot_geom(s, Ls):
    """Per edge (in eo order): supertile, tile-in-supertile, slot rank, row."""
    order, deg = s['order'], s['deg']
    pos_e = s['pos_of'][s['ldst']]
    eo = np.lexsort((np.arange(len(pos_e)), pos_e))
    pos_sorted = pos_e[eo]
    starts = np.concatenate([[0], np.cumsum(deg[order])])
    rank = np.arange(len(eo)) - starts[pos_sorted]
    t_of = pos_sorted // P
    st_of = t_of // SUPER
    p_of = pos_sorted % P
    return (eo, st_of.astype(np.int64), (t_of % SUPER).astype(np.int64),
            rank.astype(np.int64), p_of.astype(np.int64))


def pack_G(vals_bf, st_of, s_of, rank, p_of, offsC, C, Ls, totc):
    """l-innermost: col = offs[st] + s*(C*L) + c*L + rank."""
    buf = np.zeros((P, totc), BF)
    L_e = np.asarray(Ls)[st_of]
    base = np.asarray(offsC)[st_of] + s_of * (C * L_e) + rank
    for c in range(C):
        buf[p_of, base + c * L_e] = vals_bf[:, c]
    return buf


def make_npad(s, Ls, nt):
    L_t = np.repeat(np.asarray(Ls, np.int64), SUPER)
    d = s['deg'][s['order']].reshape(nt, P)
    return np.ascontiguousarray((L_t[:, None] - d).T).astype(np.float32)


def pack_local(vals, nrow, nt):
    pk = np.zeros((8 * nrow, (nt // 8) * P), BF)
    nodes = np.arange(nt * P)
    a = (nodes // P) % 8
    col = (nodes // (8 * P)) * P + nodes % P
    v = vals.astype(BF)
    for f in range(nrow - 1):
        pk[a * nrow + f, col] = v[:, f]
    pk[a * nrow + (nrow - 1), col] = BF(1.0)
    return pk


def blockdiag(w, bias, nrow, sp=16):
    bd = np.zeros((8 * nrow, 8 * sp), np.float32)
    k = w.shape[1]
    for a in range(8):
        bd[a * nrow:a * nrow + w.shape[0], a * sp:a * sp + k] = w
        bd[a * nrow + nrow - 1, a * sp:a * sp + k] = bias
    return bd.astype(BF)


def pm(vals, nt):
    d = vals.shape[1]
    return np.ascontiguousarray(
        vals.reshape(nt, P, d).transpose(1, 0, 2).reshape(P, nt * d))


def pm_pair(vals, nt):
    """[nt*P, d] -> [P, nt*d*2] with each channel duplicated (pair trick)."""
    d = vals.shape[1]
    v = vals.reshape(nt, P, d).transpose(1, 0, 2)        # [P, nt, d]
    v2 = np.repeat(v, 2, axis=2)                          # [P, nt, 2d]
    return np.ascontiguousarray(v2.reshape(P, nt * d * 2))


def attn_fold(attn_hf, H, F):
    """Per (h,f): permuted order (pos-signs first within each head),
    channel scale, prelu alpha.  Returns (perm j-list, scale, alpha)."""
    perm, scale, alpha = [], [], []
    for h in range(H):
        fs = sorted(range(F), key=lambda f: 0 if attn_hf[h, f] > 0 else 1)
        for f in fs:
            a = float(attn_hf[h, f])
            if a > 0:
                aa = max(a, 1e-8)
                perm.append(h * F + f); scale.append(aa); alpha.append(NEG_GAT)
            else:
                aa = min(a, -1e-8)
                perm.append(h * F + f); scale.append(NEG_GAT * aa); alpha.append(1.0 / NEG_GAT)
    return perm, np.asarray(scale, np.float64), alpha


def alpha_runs(alphas):
    runs = []
    i = 0
    while i < len(alphas):
        j = i
        while j < len(alphas) and alphas[j] == alphas[i]:
            j += 1
        runs.append((i, j, float(alphas[i])))
        i = j
    return runs


# ================================================================ device bits
def halving_tree(tt, X, L, out_final):
    """In-place halving over innermost axis of X [P,...,L]; final add -> out_final."""
    cur = L
    while cur > 2:
        h = cur // 2
        tt(out=X[..., 0:h], in0=X[..., 0:h], in1=X[..., cur - h:cur], op=AL.add)
        cur -= h
    if cur == 2:
        tt(out=out_final, in0=X[..., 0:1], in1=X[..., 1:2], op=AL.add)
    else:
        tt(out=out_final, in0=X[..., 0:1], in1=X[..., 0:1], op=AL.bypass)


def emit_elu(nc, sbS, hflat, nelem, tag):
    tmp = sbS.tile([P, nelem], f32, tag=tag)
    nc.vector.tensor_scalar_min(out=tmp[:], in0=hflat, scalar1=0.0)
    nc.scalar.activation(out=tmp[:], in_=tmp[:], func=AF.Exp)
    nc.vector.tensor_scalar(out=hflat, in0=hflat, scalar1=0.0, scalar2=-1.0,
                            op0=AL.max, op1=AL.add)
    nc.vector.tensor_tensor(out=hflat, in0=hflat, in1=tmp[:], op=AL.add)


def emit_gat_st_B(nc, sbG, sbZ, sbEX, sbT, d_g, off, L, S, fdp_ap, runs,
                  h1v, denv):
    """Layer-1 supertile: C=12 channels [d1h0(5), d1h1(5), a1h0, a1h1]."""
    C = 12
    L2 = L // 2
    ncols = S * C * L
    G = sbG.tile([P, ncols], bf16, tag="G")
    nc.sync.dma_start(out=G[:], in_=d_g[:, off:off + ncols])
    G4 = G[:].rearrange("p (s c l) -> p s c l", s=S, c=C, l=L)
    G5 = G[:].rearrange("p (sc l2 j) -> p sc l2 j", sc=S * C, l2=L2, j=2)
    Z = sbZ.tile([P, ncols], bf16, tag="Z")
    Z4 = Z[:].rearrange("p (s c l) -> p s c l", s=S, c=C, l=L)
    Z5 = Z[:].rearrange("p (sc l2 j) -> p sc l2 j", sc=S * C, l2=L2, j=2)
    fdb = fdp_ap.rearrange("p s c j -> p (s c) j").unsqueeze(2) \
        .broadcast_to([P, S * C, L2, 2])
    nc.vector.tensor_tensor(out=Z5, in0=G5, in1=fdb, op=AL.add)
    for (c0, c1, al) in runs:
        nc.scalar.activation(out=Z4[:, :, c0:c1, :], in_=Z4[:, :, c0:c1, :],
                             func=AF.Prelu, alpha=al)
    # score tree: d1 heads = sum of 5 channels each
    T = sbT.tile([P, S * 4 * L], bf16, tag="T")
    T4 = T[:].rearrange("p (s c l) -> p s c l", s=S, c=4, l=L)
    Tp = T[:].rearrange("p (s c2 c l) -> p s c2 c l", s=S, c2=2, c=2, l=L)
    nc.vector.tensor_tensor(out=T4[:, :, 0:2, :], in0=Z4[:, :, 0:2, :],
                            in1=Z4[:, :, 2:4, :], op=AL.add)
    nc.vector.tensor_tensor(out=T4[:, :, 2:4, :], in0=Z4[:, :, 5:7, :],
                            in1=Z4[:, :, 7:9, :], op=AL.add)
    TX = sbT.tile([P, S * 2 * L], bf16, tag="TX")
    TX3 = TX[:].rearrange("p (s c l) -> p s c l", s=S, c=2, l=L)
    nc.vector.tensor_tensor(out=TX3, in0=Tp[:, :, :, 0, :],
                            in1=Tp[:, :, :, 1, :], op=AL.add)
    SC = sbT.tile([P, S * 2 * L], bf16, tag="SC")
    SC3 = SC[:].rearrange("p (s c l) -> p s c l", s=S, c=2, l=L)
    Zh = Z4[:, :, 0:10, :].rearrange("p s (h f) l -> p s h f l", h=2, f=5)
    nc.vector.tensor_tensor(out=SC3, in0=TX3, in1=Zh[:, :, :, 4, :], op=AL.add)
    EX = sbEX.tile([P, S * 4 * L], bf16, tag="EX")
    EX4 = EX[:].rearrange("p (s c l) -> p s c l", s=S, c=4, l=L)
    nc.scalar.activation(out=EX4[:, :, 0:2, :], in_=SC3, func=AF.Exp)
    nc.scalar.activation(out=EX4[:, :, 2:4, :], in_=Z4[:, :, 10:12, :], func=AF.Exp)
    # weighted feats in-place on G (one op per d1 head: <=3 free dims)
    for h in range(2):
        Gh = G4[:, :, 5 * h:5 * h + 5, :]
        exd = EX4[:, :, h:h + 1, :].broadcast_to([P, S, 5, L])
        nc.vector.tensor_tensor(out=Gh, in0=Gh, in1=exd, op=AL.mult)
    nc.vector.tensor_tensor(out=G4[:, :, 10:12, :], in0=G4[:, :, 10:12, :],
                            in1=EX4[:, :, 2:4, :], op=AL.mult)
    halving_tree(nc.vector.tensor_tensor, G4, L, h1v.unsqueeze(3))
    halving_tree(nc.gpsimd.tensor_tensor, EX4, L, denv.unsqueeze(3))


def emit_gat_st_C(nc, sbG, sbZ, sbEX, sbT, d_g, off, L, S, fdp_ap, runs,
                  h2v, denv):
    """Layer-2 supertile: C=4 channels [d2h0(2), d2h1(2)]."""
    C = 4
    L2 = L // 2
    ncols = S * C * L
    G = sbG.tile([P, ncols], bf16, tag="G")
    nc.sync.dma_start(out=G[:], in_=d_g[:, off:off + ncols])
    G4 = G[:].rearrange("p (s c l) -> p s c l", s=S, c=C, l=L)
    G5 = G[:].rearrange("p (sc l2 j) -> p sc l2 j", sc=S * C, l2=L2, j=2)
    Z = sbZ.tile([P, ncols], bf16, tag="Z")
    Z4 = Z[:].rearrange("p (s c l) -> p s c l", s=S, c=C, l=L)
    Z5 = Z[:].rearrange("p (sc l2 j) -> p sc l2 j", sc=S * C, l2=L2, j=2)
    fdb = fdp_ap.rearrange("p s c j -> p (s c) j").unsqueeze(2) \
        .broadcast_to([P, S * C, L2, 2])
    nc.vector.tensor_tensor(out=Z5, in0=G5, in1=fdb, op=AL.add)
    for (c0, c1, al) in runs:
        nc.scalar.activation(out=Z4[:, :, c0:c1, :], in_=Z4[:, :, c0:c1, :],
                             func=AF.Prelu, alpha=al)
    Zp = Z4.rearrange("p s (h f) l -> p s h f l", h=2, f=2)
    SC = sbT.tile([P, S * 2 * L], bf16, tag="SC")
    SC3 = SC[:].rearrange("p (s c l) -> p s c l", s=S, c=2, l=L)
    nc.vector.tensor_tensor(out=SC3, in0=Zp[:, :, :, 0, :],
                            in1=Zp[:, :, :, 1, :], op=AL.add)
    EX = sbEX.tile([P, S * 2 * L], bf16, tag="EX")
    EX3 = EX[:].rearrange("p (s c l) -> p s c l", s=S, c=2, l=L)
    nc.scalar.activation(out=EX3, in_=SC3, func=AF.Exp)
    for h in range(2):
        Gh = G4[:, :, 2 * h:2 * h + 2, :]
        exd = EX3[:, :, h:h + 1, :].broadcast_to([P, S, 2, L])
        nc.vector.tensor_tensor(out=Gh, in0=Gh, in1=exd, op=AL.mult)
    halving_tree(nc.vector.tensor_tensor, G4, L, h2v.unsqueeze(3))
    halving_tree(nc.gpsimd.tensor_tensor, EX3, L, denv.unsqueeze(3))


def emit_fixup(nc, sbS, hv, hflat, dv, dflat, fd_ap, rs_flat, np_ap, isc_ap,
               T, C, NH, dF, runs, do_elu=True):
    """Pad-slot den fix + normalize + unscale + residual + elu over T tiles.
    fd_ap: [P, T, C] per-node scaled dest proj; isc_ap: [P, C] inv scales."""
    zp = sbS.tile([P, T * C], bf16, tag="zp")
    zp3 = zp[:].rearrange("p (t c) -> p t c", t=T, c=C)
    for (c0, c1, al) in runs:
        nc.scalar.activation(out=zp3[:, :, c0:c1], in_=fd_ap[:, :, c0:c1],
                             func=AF.Prelu, alpha=al)
    ep = sbS.tile([P, T * NH], bf16, tag="ep")
    ep3 = ep[:].rearrange("p (t h) -> p t h", t=T, h=NH)
    if C == 12:
        zph = zp3[:, :, 0:10].rearrange("p t (h f) -> p t h f", h=2, f=5)
        tp = sbS.tile([P, T * 2], bf16, tag="tp")
        tp3 = tp[:].rearrange("p (t h) -> p t h", t=T, h=2)
        nc.vector.tensor_tensor(out=tp3, in0=zph[:, :, :, 0], in1=zph[:, :, :, 1], op=AL.add)
        nc.vector.tensor_tensor(out=tp3, in0=tp3, in1=zph[:, :, :, 2], op=AL.add)
        nc.vector.tensor_tensor(out=tp3, in0=tp3, in1=zph[:, :, :, 3], op=AL.add)
        nc.vector.tensor_tensor(out=tp3, in0=tp3, in1=zph[:, :, :, 4], op=AL.add)
        nc.scalar.activation(out=ep3[:, :, 0:2], in_=tp3, func=AF.Exp)
        nc.scalar.activation(out=ep3[:, :, 2:4], in_=zp3[:, :, 10:12], func=AF.Exp)
    else:
        zph = zp3.rearrange("p t (h f) -> p t h f", h=2, f=2)
        tp = sbS.tile([P, T * 2], bf16, tag="tp")
        tp3 = tp[:].rearrange("p (t h) -> p t h", t=T, h=2)
        nc.vector.tensor_tensor(out=tp3, in0=zph[:, :, :, 0], in1=zph[:, :, :, 1], op=AL.add)
        nc.scalar.activation(out=ep3, in_=tp3, func=AF.Exp)
    padm = sbS.tile([P, T * NH], f32, tag="padm")
    pm3 = padm[:].rearrange("p (t h) -> p t h", t=T, h=NH)
    npb = np_ap.unsqueeze(2).broadcast_to([P, T, NH])
    nc.vector.tensor_tensor(out=pm3, in0=ep3, in1=npb, op=AL.mult)
    nc.vector.tensor_tensor(out=dv, in0=dv, in1=pm3, op=AL.subtract)
    nc.vector.tensor_scalar_max(out=dflat, in0=dflat, scalar1=1e-30)
    rec = sbS.tile([P, T * NH], f32, tag="rec")
    nc.vector.reciprocal(out=rec[:], in_=dflat)
    rec3 = rec[:].rearrange("p (t h) -> p t h", t=T, h=NH)
    # rec12 = rec[h(c)] * inv_scale_c
    rc = sbS.tile([P, T * C], f32, tag="rc")
    rc3 = rc[:].rearrange("p (t c) -> p t c", t=T, c=C)
    iscb = isc_ap.unsqueeze(1).broadcast_to([P, T, C])
    if C == 12:
        rch = rc3[:, :, 0:10].rearrange("p t (h f) -> p t h f", h=2, f=5)
        rb = rec3[:, :, 0:2].unsqueeze(3).broadcast_to([P, T, 2, 5])
        ib = iscb[:, :, 0:10].rearrange("p t (h f) -> p t h f", h=2, f=5)
        nc.vector.tensor_tensor(out=rch, in0=rb, in1=ib, op=AL.mult)
        nc.vector.tensor_tensor(out=rc3[:, :, 10:12], in0=rec3[:, :, 2:4],
                                in1=iscb[:, :, 10:12], op=AL.mult)
    else:
        rch = rc3.rearrange("p t (h f) -> p t h f", h=2, f=2)
        rb = rec3.unsqueeze(3).broadcast_to([P, T, 2, 2])
        ib = iscb.rearrange("p t (h f) -> p t h f", h=2, f=2)
        nc.vector.tensor_tensor(out=rch, in0=rb, in1=ib, op=AL.mult)
    nc.vector.tensor_tensor(out=hflat, in0=hflat, in1=rc[:], op=AL.mult)
    nc.vector.tensor_tensor(out=hflat, in0=hflat, in1=rs_flat, op=AL.add)
    if do_elu:
        emit_elu(nc, sbS, hflat, T * C, "elu")


# =============================================================== launch A
def build_launchA(nt):
    cols = nt * 16
    nc = bacc.Bacc("TRN2", target_bir_lowering=False, debug=False, num_devices=NCORE)
    d_x = nc.dram_tensor("x5l", [48, cols], bf16, kind="ExternalInput")
    d_bfs = nc.dram_tensor("bd_fs", [48, P], bf16, kind="ExternalInput")
    d_bfd = nc.dram_tensor("bd_fd", [48, P], bf16, kind="ExternalInput")
    d_brs = nc.dram_tensor("bd_rs", [48, P], bf16, kind="ExternalInput")
    d_fs = nc.dram_tensor("fs1cm", [P, cols], bf16, kind="ExternalOutput")
    d_fd = nc.dram_tensor("fd1cm", [P, cols], bf16, kind="ExternalOutput")
    d_rs = nc.dram_tensor("rs1cm", [P, cols], f32, kind="ExternalOutput")
    with TileContext(nc) as tc:
        with tc.tile_pool(name="res", bufs=1) as res, \
             tc.tile_pool(name="ps", bufs=2, space="PSUM") as ps:
            stg = res.tile([48, cols], bf16)
            nc.sync.dma_start(out=stg[:], in_=d_x[:, :])
            bfs = res.tile([48, P], bf16, tag="bfs")
            nc.sync.dma_start(out=bfs[:], in_=d_bfs[:, :])
            bfd = res.tile([48, P], bf16, tag="bfd")
            nc.sync.dma_start(out=bfd[:], in_=d_bfd[:, :])
            brs = res.tile([48, P], bf16, tag="brs")
            nc.sync.dma_start(out=brs[:], in_=d_brs[:, :])
            ofs = res.tile([P, cols], bf16, tag="ofs")
            ofd = res.tile([P, cols], bf16, tag="ofd")
            ors = res.tile([P, cols], f32, tag="ors")
            k = 0
            for j0 in range(0, cols, 512):
                w = min(512, cols - j0)
                for bd, ot in ((bfs, ofs), (bfd, ofd), (brs, ors)):
                    pmm = ps.tile([P, 512], f32, tag="mm")
                    nc.tensor.matmul(out=pmm[:, :w], lhsT=bd[:], rhs=stg[:, j0:j0 + w],
                                     start=True, stop=True)
                    if k % 2 == 0:
                        nc.vector.tensor_copy(out=ot[:, j0:j0 + w], in_=pmm[:, :w])
                    else:
                        nc.scalar.copy(out=ot[:, j0:j0 + w], in_=pmm[:, :w])
                    k += 1
            nc.sync.dma_start(out=d_fs[:, :], in_=ofs[:])
            nc.sync.dma_start(out=d_fd[:, :], in_=ofd[:])
            nc.sync.dma_start(out=d_rs[:, :], in_=ors[:])
    nc.compile()
    return nc


# =============================================================== launch B
def build_launchB(nst, Ls, offs12, nt, runs12):
    totc = int(offs12[-1])
    fgw = -(-nst // 4) * SUPER * P          # f2 output column width
    nc = bacc.Bacc("TRN2", target_bir_lowering=False, debug=False, num_devices=NCORE)
    d_g = nc.dram_tensor("g1", [P, totc], bf16, kind="ExternalInput")
    d_fdp = nc.dram_tensor("fdp1", [P, nt * 24], bf16, kind="ExternalInput")
    d_rs = nc.dram_tensor("rs1n", [P, nt * 12], f32, kind="ExternalInput")
    d_np = nc.dram_tensor("npad", [P, nt], f32, kind="ExternalInput")
    d_isc = nc.dram_tensor("isc12", [P, 12], f32, kind="ExternalInput")
    d_w2 = nc.dram_tensor("w2all", [10, 12], bf16, kind="ExternalInput")
    d_bc2 = nc.dram_tensor("bc2", [12], f32, kind="ExternalInput")
    d_f2 = nc.dram_tensor("f2cm", [48, fgw], f32, kind="ExternalOutput")
    d_ha = nc.dram_tensor("hattn", [P, nt * 2], bf16, kind="ExternalOutput")
    groups = [(0, 5), (5, 9), (9, nst)] if nst >= 9 else [(0, nst)]
    with TileContext(nc) as tc:
        with tc.tile_pool(name="res", bufs=1) as res, \
             tc.tile_pool(name="sbG", bufs=2) as sbG, \
             tc.tile_pool(name="sbZ", bufs=2) as sbZ, \
             tc.tile_pool(name="sbEX", bufs=2) as sbEX, \
             tc.tile_pool(name="sbT", bufs=1) as sbT, \
             tc.tile_pool(name="sbS", bufs=1) as sbS, \
             tc.tile_pool(name="sbT2", bufs=2) as sbT2, \
             tc.tile_pool(name="psT", bufs=1, space="PSUM") as psT, \
             tc.tile_pool(name="psF", bufs=1, space="PSUM") as psF:
            ident = res.tile([P, P], bf16)
            make_identity(nc, ident[:])
            npad = res.tile([P, nt], f32)
            nc.sync.dma_start(out=npad[:], in_=d_np[:, :])
            isc = res.tile([P, 12], f32)
            nc.sync.dma_start(out=isc[:], in_=d_isc[:, :])
            fdp = res.tile([P, nt * 24], bf16)
            nc.sync.dma_start(out=fdp[:], in_=d_fdp[:, :])
            rst = res.tile([P, nt * 12], f32)
            nc.sync.dma_start(out=rst[:], in_=d_rs[:, :])
            w2t = res.tile([10, 12], bf16, tag="w2t")
            nc.sync.dma_start(out=w2t[:], in_=d_w2[:, :])
            bc2 = res.tile([12, 1], f32, tag="bc2")
            nc.sync.dma_start(out=bc2[:], in_=d_bc2[:, None])
            h1 = res.tile([P, nt * 12], f32)
            den = res.tile([P, nt * 4], f32)
            hat = res.tile([P, nt * 2], bf16)
            h1v_all = h1[:].rearrange("p (t c) -> p t c", t=nt, c=12)
            denv_all = den[:].rearrange("p (t c) -> p t c", t=nt, c=4)
            fdp_all = fdp[:].rearrange("p (t c j) -> p t c j", t=nt, c=12, j=2)
            npv_all = npad[:].rearrange("p (t o) -> p t o", t=nt, o=1)
            for g0, g1 in groups:
                for st in range(g0, g1):
                    L = Ls[st]
                    t0 = st * SUPER
                    emit_gat_st_B(nc, sbG, sbZ, sbEX, sbT, d_g, int(offs12[st]),
                                  L, SUPER, fdp_all[:, t0:t0 + SUPER],
                                  runs12,
                                  h1v_all[:, t0:t0 + SUPER, :],
                                  denv_all[:, t0:t0 + SUPER, :])
                T = (g1 - g0) * SUPER
                t0 = g0 * SUPER
                emit_fixup(nc, sbS, h1v_all[:, t0:t0 + T, :],
                           h1[:, t0 * 12:(t0 + T) * 12],
                           denv_all[:, t0:t0 + T, :],
                           den[:, t0 * 4:(t0 + T) * 4],
                           fdp_all[:, t0:t0 + T, :, 0],
                           rst[:, t0 * 12:(t0 + T) * 12],
                           npv_all[:, t0:t0 + T, 0], isc[:], T, 12, 4, 5, runs12)
                hv = h1v_all[:, t0:t0 + T, :]
                nc.scalar.copy(out=hat[:].rearrange("p (t c) -> p t c", t=nt, c=2)[:, t0:t0 + T, :],
                               in_=hv[:, :, 10:12])
                h1b = sbS.tile([P, T * 10], bf16, tag="h1b")
                nc.scalar.copy(out=h1b[:].rearrange("p (t c) -> p t c", t=T, c=10),
                               in_=hv[:, :, 0:10])
                # epilogue: transpose h_def1 per tile, project fs2''|fd2''|res2
                for st in range(g0, g1):
                    pT = psT.tile([10, SUPER * P], bf16, tag="pT")
                    for b in range(SUPER):
                        trel = (st - g0) * SUPER + b
                        nc.tensor.transpose(out=pT[:, b * P:(b + 1) * P],
                                            in_=h1b[:, trel * 10:trel * 10 + 10],
                                            identity=ident[:])
                    hT = sbT2.tile([10, SUPER * P], bf16, tag="hT")
                    nc.vector.tensor_copy(out=hT[:], in_=pT[:])
                    pF = psF.tile([12, SUPER * P], f32, tag="pF")
                    for q in range(SUPER * P // 512):
                        nc.tensor.matmul(out=pF[:, q * 512:(q + 1) * 512], lhsT=w2t[:],
                                         rhs=hT[:, q * 512:(q + 1) * 512],
                                         start=True, stop=True)
                    f2s = sbT2.tile([12, SUPER * P], f32, tag="f2s")
                    nc.scalar.activation(out=f2s[:], in_=pF[:], func=AF.Prelu,
                                         alpha=1.0, bias=bc2[:])
                    nc.sync.dma_start(
                        out=d_f2[12 * (st % 4):12 * (st % 4) + 12,
                                 (st // 4) * SUPER * P:(st // 4 + 1) * SUPER * P],
                        in_=f2s[:])
            nc.sync.dma_start(out=d_ha[:, :], in_=hat[:])
    nc.compile()
    return nc


# =============================================================== launch C
def build_launchC(nst, Ls, offs4, nt, nmc, runs4):
    totc = int(offs4[-1])
    ngrp = -(-nmc // 4)
    nc = bacc.Bacc("TRN2", target_bir_lowering=False, debug=False, num_devices=NCORE)
    d_g = nc.dram_tensor("g2", [P, totc], bf16, kind="ExternalInput")
    d_fdp = nc.dram_tensor("fdp2", [P, nt * 8], bf16, kind="ExternalInput")
    d_rs = nc.dram_tensor("rs2n", [P, nt * 4], f32, kind="ExternalInput")
    d_np = nc.dram_tensor("npad", [P, nt], f32, kind="ExternalInput")
    d_isc = nc.dram_tensor("isc4", [P, 4], f32, kind="ExternalInput")
    d_ha = nc.dram_tensor("hattn", [P, nt * 2], bf16, kind="ExternalInput")
    d_x = nc.dram_tensor("xpm", [P, nt * 8], bf16, kind="ExternalInput")
    d_w1 = nc.dram_tensor("w1x", [32, 196], bf16, kind="ExternalInput")
    w2dt = fp8 if FP8_W2 else bf16
    d_w2a = nc.dram_tensor("w2dra", [P, 2 * 128], w2dt, kind="ExternalInput")
    d_w2b = nc.dram_tensor("w2drb", [P, 2 * 128], w2dt, kind="ExternalInput")
    d_w3a = nc.dram_tensor("w3a", [P, 14], bf16, kind="ExternalInput")
    d_w3b = nc.dram_tensor("w3b", [68, 14], bf16, kind="ExternalInput")
    d_w4 = nc.dram_tensor("w4blk", [110, 4], bf16, kind="ExternalInput")
    d_b1 = nc.dram_tensor("b1", [196], f32, kind="ExternalInput")
    d_b2 = nc.dram_tensor("b2", [196], f32, kind="ExternalInput")
    d_b3 = nc.dram_tensor("b3", [14], f32, kind="ExternalInput")
    d_b4 = nc.dram_tensor("b4r", [4], f32, kind="ExternalInput")
    d_out = nc.dram_tensor("out", [nmc, 512], f32, kind="ExternalOutput")
    groups = [(0, 3), (3, 6), (6, 9), (9, 11), (11, nst)] if nst >= 11 else [(0, nst)]
    with TileContext(nc) as tc:
        with tc.tile_pool(name="res", bufs=1) as res, \
             tc.tile_pool(name="sbG", bufs=2) as sbG, \
             tc.tile_pool(name="sbZ", bufs=2) as sbZ, \
             tc.tile_pool(name="sbEX", bufs=2) as sbEX, \
             tc.tile_pool(name="sbT", bufs=1) as sbT, \
             tc.tile_pool(name="sbS", bufs=1) as sbS, \
             tc.tile_pool(name="sbM", bufs=2) as sbM, \
             tc.tile_pool(name="psT", bufs=1, space="PSUM") as psT, \
             tc.tile_pool(name="psC", bufs=1, space="PSUM") as psC, \
             tc.tile_pool(name="psA", bufs=2, space="PSUM") as psA, \
             tc.tile_pool(name="psB", bufs=2, space="PSUM") as psB, \
             tc.tile_pool(name="psO", bufs=2, space="PSUM") as psO:
            ident = res.tile([P, P], bf16)
            make_identity(nc, ident[:])
            npad = res.tile([P, nt], f32)
            nc.sync.dma_start(out=npad[:], in_=d_np[:, :])
            isc = res.tile([P, 4], f32)
            nc.sync.dma_start(out=isc[:], in_=d_isc[:, :])
            fdp = res.tile([P, nt * 8], bf16)
            nc.sync.dma_start(out=fdp[:], in_=d_fdp[:, :])
            rst = res.tile([P, nt * 4], f32)
            nc.sync.dma_start(out=rst[:], in_=d_rs[:, :])
            hat = res.tile([P, nt * 2], bf16)
            nc.sync.dma_start(out=hat[:], in_=d_ha[:, :])
            xpm = res.tile([P, nt * 8], bf16)
            nc.sync.dma_start(out=xpm[:], in_=d_x[:, :])
            w1 = res.tile([32, 196], bf16, tag="w1")
            nc.sync.dma_start(out=w1[:], in_=d_w1[:, :])
            w2a = res.tile([P, 2 * 128], w2dt, tag="w2a")
            nc.sync.dma_start(out=w2a[:], in_=d_w2a[:, :])
            w2b = res.tile([P, 2 * 128], w2dt, tag="w2b")
            nc.sync.dma_start(out=w2b[:], in_=d_w2b[:, :])
            w3a = res.tile([P, 14], bf16, tag="w3a")
            nc.sync.dma_start(out=w3a[:], in_=d_w3a[:, :])
            w3b = res.tile([68, 14], bf16, tag="w3b")
            nc.sync.dma_start(out=w3b[:], in_=d_w3b[:, :])
            w4b = res.tile([110, 4], bf16, tag="w4b")
            nc.sync.dma_start(out=w4b[:], in_=d_w4[:, :])
            w2av = w2a[:].rearrange("p (k m) -> p k m", k=2, m=128)
            w2bv = w2b[:].rearrange("p (k m) -> p k m", k=2, m=128)
            b1ca = res.tile([P, 1], f32, tag="b1ca")
            nc.sync.dma_start(out=b1ca[:], in_=d_b1[0:128, None])
            b1cb = res.tile([68, 1], f32, tag="b1cb")
            nc.sync.dma_start(out=b1cb[:], in_=d_b1[128:196, None])
            b2ca = res.tile([P, 1], f32, tag="b2ca")
            nc.sync.dma_start(out=b2ca[:], in_=d_b2[0:128, None])
            b2cb = res.tile([68, 1], f32, tag="b2cb")
            nc.sync.dma_start(out=b2cb[:], in_=d_b2[128:196, None])
            b3c = res.tile([14, 1], f32, tag="b3c")
            nc.sync.dma_start(out=b3c[:], in_=d_b3[:, None])
            b4c = res.tile([4, 1], f32, tag="b4c")
            nc.sync.dma_start(out=b4c[:], in_=d_b4[:, None])
            h2 = res.tile([P, nt * 4], f32)
            den = res.tile([P, nt * 2], f32)
            m32 = res.tile([P, nt * 32], bf16)
            nc.gpsimd.memset(m32[:], 0.0)
            r3st = res.tile([110, ngrp * 512], bf16, tag="r3st")
            adt = fp8 if FP8_W2 else bf16
            r1t = [res.tile([P, 2 * 512], adt, tag=f"r1_{i}", name=f"r1_{i}")
                   for i in range(2)]
            for t_ in r1t:
                nc.gpsimd.memset(t_[:], 0.0)
            h2v_all = h2[:].rearrange("p (t c) -> p t c", t=nt, c=4)
            denv_all = den[:].rearrange("p (t c) -> p t c", t=nt, c=2)
            fdp_all = fdp[:].rearrange("p (t c j) -> p t c j", t=nt, c=4, j=2)
            npv_all = npad[:].rearrange("p (t o) -> p t o", t=nt, o=1)
            m3_all = m32[:].rearrange("p (t c) -> p t c", t=nt, c=32)
            hav_all = hat[:].rearrange("p (t c) -> p t c", t=nt, c=2)
            xv_all = xpm[:].rearrange("p (t c) -> p t c", t=nt, c=8)
            nc.vector.tensor_copy(out=m3_all[:, :, 0:2], in_=hav_all)
            nc.vector.tensor_copy(out=m3_all[:, :, 6:14], in_=xv_all)
            for g0, g1 in groups:
                for st in range(g0, g1):
                    L = Ls[st]
                    t0 = st * SUPER
                    emit_gat_st_C(nc, sbG, sbZ, sbEX, sbT, d_g, int(offs4[st]),
                                  L, SUPER, fdp_all[:, t0:t0 + SUPER],
                                  runs4,
                                  h2v_all[:, t0:t0 + SUPER, :],
                                  denv_all[:, t0:t0 + SUPER, :])
                T = (g1 - g0) * SUPER
                t0 = g0 * SUPER
                emit_fixup(nc, sbS, h2v_all[:, t0:t0 + T, :],
                           h2[:, t0 * 4:(t0 + T) * 4],
                           denv_all[:, t0:t0 + T, :],
                           den[:, t0 * 2:(t0 + T) * 2],
                           fdp_all[:, t0:t0 + T, :, 0],
                           rst[:, t0 * 4:(t0 + T) * 4],
                           npv_all[:, t0:t0 + T, 0], isc[:], T, 4, 2, 2, runs4)
                nc.vector.tensor_copy(out=m3_all[:, t0:t0 + T, 2:6],
                                      in_=h2v_all[:, t0:t0 + T, :])
                # MLP over this group's chunks (4 tiles = 512 nodes each)
                for mc in range(t0 // 4, min((t0 + T) // 4, nmc)):
                    pT = psT.tile([64, 256], bf16, tag="pT")
                    nc.tensor.transpose(out=pT[:, 0:128],
                                        in_=m32[:, (mc * 4) * 32:(mc * 4 + 2) * 32],
                                        identity=ident[:])
                    nc.tensor.transpose(out=pT[:, 128:256],
                                        in_=m32[:, (mc * 4 + 2) * 32:(mc * 4 + 4) * 32],
                                        identity=ident[:])
                    r0 = sbM.tile([32, 512], bf16, tag="r0")
                    nc.vector.tensor_copy(out=r0[:, 0:128], in_=pT[0:32, 0:128])
                    nc.vector.tensor_copy(out=r0[:, 128:256], in_=pT[32:64, 0:128])
                    nc.vector.tensor_copy(out=r0[:, 256:384], in_=pT[0:32, 128:256])
                    nc.vector.tensor_copy(out=r0[:, 384:512], in_=pT[32:64, 128:256])
                    p1a = psA.tile([P, 512], f32, tag="pA")
                    nc.tensor.matmul(out=p1a[:], lhsT=w1[:, 0:128], rhs=r0[:], start=True, stop=True)
                    p1b = psB.tile([P, 512], f32, tag="pB")
                    nc.tensor.matmul(out=p1b[0:68, :], lhsT=w1[:, 128:196], rhs=r0[:], start=True, stop=True)
                    r1 = r1t[mc % 2]
                    r1v = r1[:].rearrange("p (k n) -> p k n", k=2, n=512)
                    nc.scalar.activation(out=r1v[:, 0:1, :], in_=p1a[:].unsqueeze(1),
                                         func=AF.Prelu, alpha=NEG_MLP, bias=b1ca[:])
                    nc.scalar.activation(out=r1v[0:68, 1:2, :], in_=p1b[0:68, :].unsqueeze(1),
                                         func=AF.Prelu, alpha=NEG_MLP, bias=b1cb[:])
                    p2a = psA.tile([P, 512], f32, tag="pA")
                    p2b = psB.tile([P, 512], f32, tag="pB")
                    if FP8_W2:
                        nc.tensor.matmul(out=p2a[:], lhsT=w2av, rhs=r1v,
                                         start=True, stop=True, perf_mode=MPM.DoubleRow)
                        nc.tensor.matmul(out=p2b[:], lhsT=w2bv, rhs=r1v,
                                         start=True, stop=True, perf_mode=MPM.DoubleRow)
                    else:
                        nc.tensor.matmul(out=p2a[:], lhsT=w2av[:, 0, :], rhs=r1v[:, 0, :], start=True, stop=False)
                        nc.tensor.matmul(out=p2a[:], lhsT=w2av[0:68, 1, :], rhs=r1v[0:68, 1, :], start=False, stop=True)
                        nc.tensor.matmul(out=p2b[:], lhsT=w2bv[:, 0, :], rhs=r1v[:, 0, :], start=True, stop=False)
                        nc.tensor.matmul(out=p2b[:], lhsT=w2bv[0:68, 1, :], rhs=r1v[0:68, 1, :], start=False, stop=True)
                    r2a = sbM.tile([P, 512], bf16, tag="r2a")
                    nc.scalar.activation(out=r2a[:], in_=p2a[:], func=AF.Prelu,
                                         alpha=NEG_MLP, bias=b2ca[:])
                    r2b = sbM.tile([68, 512], bf16, tag="r2b")
                    nc.scalar.activation(out=r2b[:], in_=p2b[0:68, :], func=AF.Prelu,
                                         alpha=NEG_MLP, bias=b2cb[:])
                    p3 = psC.tile([14, 512], f32, tag="p3")
                    nc.tensor.matmul(out=p3[:], lhsT=w3a[:], rhs=r2a[:], start=True, stop=False)
                    nc.tensor.matmul(out=p3[:], lhsT=w3b[:], rhs=r2b[:], start=False, stop=True)
                    ro = 32 * (mc % 4)
                    nc.scalar.activation(out=r3st[ro:ro + 14, (mc // 4) * 512:(mc // 4 + 1) * 512],
                                         in_=p3[:], func=AF.Prelu,
                                         alpha=NEG_MLP, bias=b3c[:])
            # tail: stacked 14->1 matmuls + sigmoids (one act-table switch)
            for g in range(ngrp):
                k = min(4, nmc - 4 * g)
                kp = 32 * (k - 1) + 14
                po = psO.tile([4, 512], f32, tag="po")
                nc.tensor.matmul(out=po[0:k, :], lhsT=w4b[0:kp, 0:k],
                                 rhs=r3st[0:kp, g * 512:(g + 1) * 512],
                                 start=True, stop=True)
                sg = sbM.tile([4, 512], f32, tag="sg")
                nc.scalar.activation(out=sg[0:k, :], in_=po[0:k, :], func=AF.Sigmoid,
                                     bias=b4c[0:k, :])
                nc.sync.dma_start(out=d_out[4 * g:4 * g + k, :], in_=sg[0:k, :])
    nc.compile()
    return nc


# ================================================================== kernel
_cache = {}


def kernel(**inputs):
    x = np.asarray(inputs['x'], np.float32)
    src = np.asarray(inputs['src'], np.int32)
    dst = np.asarray(inputs['dst'], np.int32)
    n = x.shape[0]

    scheds, nst, Ls = build_schedule(dst, n)
    nt = scheds[0]['nt']
    nloc = scheds[0]['nloc']
    nmc = -(-nloc // 512)
    offs12 = np.concatenate([[0], np.cumsum([SUPER * L * 12 for L in Ls])]).astype(np.int64)
    offs4 = np.concatenate([[0], np.cumsum([SUPER * L * 4 for L in Ls])]).astype(np.int64)

    # ---- layer-1 attn folding: channels [d1h0(5), d1h1(5), a1h0, a1h1]
    d1_attn = np.asarray(inputs['d1_attn'], np.float64)     # [2, 5]
    a1_attn = np.asarray(inputs['a1_attn'], np.float64)     # [2, 1]
    perm_d1, scale_d1, alpha_d1 = attn_fold(d1_attn, 2, 5)
    perm_a1, scale_a1, alpha_a1 = attn_fold(a1_attn[:, :], 2, 1)
    scale12 = np.concatenate([scale_d1, scale_a1])
    alpha12 = alpha_d1 + alpha_a1
    runs12 = alpha_runs(alpha12)

    def l1_pack(a1_w, d1_w, scale=None):
        w = np.zeros((a1_w.shape[0], 12), np.float64)
        for p_, j in enumerate(perm_d1):
            w[:, p_] = d1_w[:, j]
        for p_, j in enumerate(perm_a1):
            w[:, 10 + p_] = a1_w[:, j]
        if scale is not None:
            w = w * scale[None, :]
        return w

    a1_Wsrc = np.asarray(inputs['a1_Wsrc'], np.float64)
    d1_Wsrc = np.asarray(inputs['d1_Wsrc'], np.float64)
    a1_Wdst = np.asarray(inputs['a1_Wdst'], np.float64)
    d1_Wdst = np.asarray(inputs['d1_Wdst'], np.float64)
    a1_Wres = np.asarray(inputs['a1_Wres'], np.float64)
    d1_Wres = np.asarray(inputs['d1_Wres'], np.float64)
    bY = l1_pack(np.asarray(inputs['a1_bsrc'], np.float64)[None, :],
                 np.asarray(inputs['d1_bsrc'], np.float64)[None, :], scale12)[0]
    bD = l1_pack(np.asarray(inputs['a1_bdst'], np.float64)[None, :],
                 np.asarray(inputs['d1_bdst'], np.float64)[None, :], scale12)[0]
    bR = l1_pack(np.asarray(inputs['a1_bias'], np.float64)[None, :],
                 np.asarray(inputs['d1_bias'], np.float64)[None, :])[0]
    bd_fs = blockdiag(l1_pack(a1_Wsrc, d1_Wsrc, scale12).astype(np.float32), bY.astype(np.float32), 6)
    bd_fd = blockdiag(l1_pack(a1_Wdst, d1_Wdst, scale12).astype(np.float32), bD.astype(np.float32), 6)
    bd_rs = blockdiag(l1_pack(a1_Wres, d1_Wres).astype(np.float32), bR.astype(np.float32), 6)
    isc12 = np.tile((1.0 / scale12).astype(np.float32), (P, 1))

    # ---- layer-2 attn folding: channels [d2h0(2), d2h1(2)]
    d2_attn = np.asarray(inputs['d2_attn'], np.float64)     # [2, 2]
    perm_d2, scale4, alpha4 = attn_fold(d2_attn, 2, 2)
    runs4 = alpha_runs(alpha4)

    def d2w(name):
        w = np.asarray(inputs[name], np.float64)            # [10, 4] native cols j=2h+f
        out = np.zeros((10, 4), np.float64)
        for p_, j in enumerate(perm_d2):
            out[:, p_] = w[:, j]
        return out

    def d2b(name):
        b = np.asarray(inputs[name], np.float64)
        return b[perm_d2]

    # rows of the [10, 12] projection are h_def1 in MY permuted order
    rowperm = perm_d1                                       # position -> native j=5h+f
    ws2 = d2w('d2_Wsrc')[rowperm] * scale4[None, :]
    wd2 = d2w('d2_Wdst')[rowperm] * scale4[None, :]
    wr2 = d2w('d2_Wres')[rowperm]
    w2all = np.concatenate([ws2, wd2, wr2], axis=1).astype(np.float32)
    bc2 = np.concatenate([d2b('d2_bsrc') * scale4, d2b('d2_bdst') * scale4,
                          d2b('d2_bias')]).astype(np.float32)
    isc4 = np.tile((1.0 / scale4).astype(np.float32), (P, 1))

    # ---- MLP weights: W1 rows 2:6 permuted to h_def2 order
    w1p = np.asarray(inputs['W1'], np.float64).copy()
    W1n = np.asarray(inputs['W1'], np.float64)
    for p_, j in enumerate(perm_d2):
        w1p[2 + p_] = W1n[2 + j]
    w1x = np.zeros((32, 196), np.float32)
    w1x[0:14] = w1p.astype(np.float32)
    W2 = np.asarray(inputs['W2'], np.float32)
    w2dra = np.zeros((P, 2, 128), np.float32)
    w2dra[:, 0, :] = W2[0:128, 0:128]
    w2dra[0:68, 1, :] = W2[128:196, 0:128]
    w2drb = np.zeros((P, 2, 128), np.float32)
    w2drb[:, 0, 0:68] = W2[0:128, 128:196]
    w2drb[0:68, 1, 0:68] = W2[128:196, 128:196]
    FPW = FP8 if FP8_W2 else BF
    w4 = np.asarray(inputs['W4'], np.float32)               # [14, 1]
    w4blk = np.zeros((110, 4), np.float32)
    for k in range(4):
        w4blk[32 * k:32 * k + 14, k] = w4[:, 0]
    b4r = np.full(4, float(np.asarray(inputs['b4'])[0]), np.float32)

    key = (n, len(src), nst, tuple(Ls), tuple(runs12), tuple(runs4))
    if key not in _cache:
        _cache.clear()
        _cache[key] = (build_launchA(nt), build_launchB(nst, Ls, offs12, nt, runs12),
                       build_launchC(nst, Ls, offs4, nt, nmc, runs4))
    ncA, ncB, ncC = _cache[key]

    # ---------------- launch A: per-node projections of x
    inA = []
    for s in scheds:
        orig = s['order']
        valid = orig < nloc
        xl = np.zeros((nt * P, 5), np.float32)
        xl[valid] = x[s['core'] * nloc + orig[valid], :5]
        inA.append(dict(x5l=pack_local(xl, 6, nt), bd_fs=bd_fs, bd_fd=bd_fd, bd_rs=bd_rs))
    rA = run_bass_kernel_spmd(ncA, inA, core_ids=list(range(NCORE)))
    tA = rA.exec_time_ns or 0

    i_all = np.arange(nt * P)
    a_i = (i_all // P) % 8
    col_i = (i_all // (8 * P)) * P + i_all % P
    rows12 = a_i[:, None] * 16 + np.arange(12)[None, :]
    fs1g = np.zeros((n, 12), BF)
    geoms, fdp1_l, rs1n_l, npad_l = [], [], [], []
    for ci, s in enumerate(scheds):
        fs_sorted = rA.results[ci]['fs1cm'][rows12, col_i[:, None]]
        fd_sorted = rA.results[ci]['fd1cm'][rows12, col_i[:, None]]
        rs_sorted = rA.results[ci]['rs1cm'][rows12, col_i[:, None]]
        orig = s['order']
        valid = orig < nloc
        fs1g[s['core'] * nloc + orig[valid]] = fs_sorted[valid]
        fdp1_l.append(pm_pair(fd_sorted, nt))
        rs1n_l.append(pm(rs_sorted.astype(np.float32), nt))
        geoms.append(edge_slot_geom(s, Ls))
        npad_l.append(make_npad(s, Ls, nt))

    inB = []
    for ci, s in enumerate(scheds):
        eo, st_of, s_of, rank, p_of = geoms[ci]
        v = fs1g[src[s['em']][eo]]
        g1 = pack_G(v, st_of, s_of, rank, p_of, offs12, 12, Ls, int(offs12[-1]))
        inB.append(dict(g1=g1, fdp1=fdp1_l[ci], rs1n=rs1n_l[ci], isc12=isc12,
                        npad=npad_l[ci], w2all=w2all.astype(BF), bc2=bc2))
    rB = run_bass_kernel_spmd(ncB, inB, core_ids=list(range(NCORE)))
    tB = rB.exec_time_ns or 0

    fgw = -(-nst // 4) * SUPER * P
    fs2g = np.zeros((n, 4), BF)
    fdp2_l, rs2n_l, ha_l, xpm_l = [], [], [], []
    for ci, s in enumerate(scheds):
        fb = rB.results[ci]['f2cm']              # [48, fgw]
        f2 = np.zeros((12, nt * P), np.float32)
        for st in range(nst):
            f2[:, st * SUPER * P:(st + 1) * SUPER * P] = \
                fb[12 * (st % 4):12 * (st % 4) + 12,
                   (st // 4) * SUPER * P:(st // 4 + 1) * SUPER * P]
        orig = s['order']
        valid = orig < nloc
        fs2g[s['core'] * nloc + orig[valid]] = f2[0:4, :].T[valid].astype(BF)
        fdp2_l.append(pm_pair(f2[4:8, :].T.astype(BF), nt))
        rs2n_l.append(pm(np.ascontiguousarray(f2[8:12, :].T), nt))
        ha_l.append(rB.results[ci]['hattn'])
        xl8 = np.zeros((nt * P, 8), np.float32)
        xl8[valid] = x[s['core'] * nloc + orig[valid], :]
        xpm_l.append(pm(xl8, nt).astype(BF))

    inC = []
    for ci, s in enumerate(scheds):
        eo, st_of, s_of, rank, p_of = geoms[ci]
        v = fs2g[src[s['em']][eo]]
        g2 = pack_G(v, st_of, s_of, rank, p_of, offs4, 4, Ls, int(offs4[-1]))
        inC.append(dict(g2=g2, fdp2=fdp2_l[ci], rs2n=rs2n_l[ci], isc4=isc4,
                        npad=npad_l[ci], hattn=ha_l[ci], xpm=xpm_l[ci],
                        w1x=w1x.astype(BF),
                        w2dra=w2dra.reshape(P, 256).astype(FPW),
                        w2drb=w2drb.reshape(P, 256).astype(FPW),
                        w3a=np.asarray(inputs['W3'], np.float32)[0:128].astype(BF),
                        w3b=np.asarray(inputs['W3'], np.float32)[128:196].astype(BF),
                        w4blk=w4blk.astype(BF),
                        b1=np.asarray(inputs['b1'], np.float32),
                        b2=np.asarray(inputs['b2'], np.float32),
                        b3=np.asarray(inputs['b3'], np.float32),
                        b4r=b4r))
    rC = run_bass_kernel_spmd(ncC, inC, core_ids=list(range(NCORE)))
    tC = rC.exec_time_ns or 0

    out = np.zeros((n, 1), np.float32)
    for ci, s in enumerate(scheds):
        y = rC.results[ci]['out'].reshape(nmc * 512)
        orig = s['order']
        valid = orig < nloc
        idx = np.arange(nt * P)[valid]
        out[s['core'] * nloc + orig[valid], 0] = y[idx]
    kernel.last_exec_ns = tA + tB + tC
    kernel.last_t12 = (tA, tB, tC)
    kernel.last_results = (rA, rB, rC)
    return out
